# revision 2
# baseline (speedup 1.0000x reference)
"""Trainium2 Bass kernel for GQA flash-attention partials (sparse_attention).

Problem: query [2,2048,16,128] f32, key/value [2,2048,8,128] f32,
decoder_segment_ids [2,2048] int32 (sorted), num_kv_heads=8.
Returns (local_out [b,t,n,d], local_max [b,t,n,1], local_sum [b,t,n,1]):
  S = einsum('btkgd,bskd->bkgts', q, k) + additive mask (segment & causal)
  local_max = rowmax(S); local_exps = 2.71828**(S - max); local_sum = rowsum
  local_out = einsum('bkgts,bskd->btkgd', local_exps, v)

Sharding: 8 cores; core c -> batch c//4, kv heads {2*(c%4), 2*(c%4)+1}
(4 query heads per core). No collectives needed; gather on host.

Per-core device pipeline (orientation: S[t_partitions, s_free]):
  - host pre-transposes Q,K to [d, t] layout; V stays [s, d] (cast to bf16)
  - per (pair, t-block): QK matmuls (float32r) -> PSUM strip [128, W]
    where the strip [s_lo, t_end) skips earlier segments (block-aligned)
  - DVE tensor_tensor_reduce: masked = S + mask(bf16), accum rowmax
  - ScalarE activation(Exp, bias=-ln(2.71828)*max, scale=ln(2.71828)),
    accum_out = rowsum; P written bf16
  - PE transpose of P chunks + DVE/ACT copies -> PT buffer (bf16)
  - PV: out^T[d, t] accumulated in PSUM over s-chunks (bf16 matmuls)
  - host transposes out^T back
"""

import numpy as np
import ml_dtypes

import concourse.bass as bass
import concourse.tile as tile
import concourse.mybir as mybir
from concourse import bacc
from concourse.bass_utils import run_bass_kernel_spmd
from concourse.masks import make_identity

B, T, NH, NKV, D = 2, 2048, 16, 8, 128
G = NH // NKV            # 2 query heads per kv head
NCORES = 8
KVC = NKV // 4           # 2 kv heads per core
PAIRS = KVC * G          # 4 query heads per core
NTB = T // 128           # 16 t-blocks
PIECE = 1024             # max PSUM strip piece width (2 banks f32)

LN = float(np.log(np.float64(np.float32(2.71828))))   # exp base scale
MASK_VAL = float(-0.7 * np.finfo(np.float32).max)
NEG_BIG = -3.0e38

F32 = mybir.dt.float32
F32R = mybir.dt.float32r
BF16 = mybir.dt.bfloat16
BF_NP = ml_dtypes.bfloat16


def _plan(seg_ids):
    """Strip extents per t-block, unioned over batches (graph is SPMD-shared).

    Returns dict with per-tb s_lo (128-aligned), W, PT-buffer column offsets
    per (tb, sc) chunk, and total column count SUMW.
    """
    seg_ids = np.asarray(seg_ids)
    # first index of the segment containing row r, per batch
    s_lo = []
    for tb in range(NTB):
        r0 = tb * 128
        lo = r0
        for b in range(seg_ids.shape[0]):
            seg = seg_ids[b]
            first = int(np.searchsorted(seg, seg[r0], side="left"))
            lo = min(lo, first)
        s_lo.append((lo // 128) * 128)
    W = [(tb + 1) * 128 - s_lo[tb] for tb in range(NTB)]
    pt_off = {}
    off = 0
    for tb in range(NTB):
        for sc in range(s_lo[tb] // 128, tb + 1):
            pt_off[(tb, sc)] = off
            off += 128
    mask_off = {}
    moff = 0
    for tb in range(NTB):
        mask_off[tb] = moff
        moff += W[tb]
    assert moff == off
    return dict(s_lo=s_lo, W=W, pt_off=pt_off, mask_off=mask_off, SUMW=moff)


def _masks_for_batch(seg, plan):
    """[128, SUMW] additive mask (0 / MASK_VAL) in bf16 for one batch."""
    seg = np.asarray(seg)
    cols = np.empty((128, plan["SUMW"]), dtype=np.float32)
    for tb in range(NTB):
        lo, w = plan["s_lo"][tb], plan["W"][tb]
        rows = np.arange(tb * 128, (tb + 1) * 128)
        cc = np.arange(lo, lo + w)
        ok = (seg[cc][None, :] == seg[rows][:, None]) & (cc[None, :] <= rows[:, None])
        cols[:, plan["mask_off"][tb]:plan["mask_off"][tb] + w] = np.where(ok, 0.0, MASK_VAL)
    return cols.astype(BF_NP)


def _build_graph(plan):
    nc = bacc.Bacc("TRN2", target_bir_lowering=False, debug=False, num_devices=NCORES)

    qt_d = nc.declare_dram_parameter("qt", [PAIRS, D, T], F32R, isOutput=False)
    kt_d = nc.declare_dram_parameter("kt", [KVC, D, T], F32R, isOutput=False)
    v_d = nc.declare_dram_parameter("v", [KVC, T, D], BF16, isOutput=False)
    mask_d = nc.declare_dram_parameter("mask", [128, plan["SUMW"]], BF16, isOutput=False)
    o_d = nc.declare_dram_parameter("o", [PAIRS, D, T], F32, isOutput=True)
    om_d = nc.declare_dram_parameter("om", [PAIRS, NTB, 128], F32, isOutput=True)
    os_d = nc.declare_dram_parameter("os", [PAIRS, NTB, 128], F32, isOutput=True)

    s_lo, W, pt_off, mask_off = plan["s_lo"], plan["W"], plan["pt_off"], plan["mask_off"]
    SUMW = plan["SUMW"]
    pt_bufs = 2 if SUMW <= 12 * 1024 else 1

    with tile.TileContext(nc) as tc:
        with (
            tc.tile_pool(name="singles", bufs=1) as singles,
            tc.tile_pool(name="ptbuf", bufs=pt_bufs) as ptbuf_pool,
            tc.tile_pool(name="p", bufs=2) as p_pool,
            tc.tile_pool(name="stats", bufs=8) as stats_pool,
            tc.tile_pool(name="stage", bufs=4) as stage_pool,
            tc.tile_pool(name="otsb", bufs=2) as otsb_pool,
            tc.tile_pool(name="sps", bufs=2, space="PSUM") as sps_pool,
            tc.tile_pool(name="ptps", bufs=2, space="PSUM") as ptps_pool,
            tc.tile_pool(name="otps", bufs=2, space="PSUM") as otps_pool,
        ):
            ident = singles.tile([128, 128], BF16)
            make_identity(nc, ident)

            qt_sb = singles.tile([128, PAIRS, T], F32R)
            nc.sync.dma_start(out=qt_sb, in_=qt_d.ap().rearrange("h d t -> d h t"))
            kt_sb = singles.tile([128, KVC, T], F32R)
            nc.sync.dma_start(out=kt_sb, in_=kt_d.ap().rearrange("h d t -> d h t"))
            v_sb = singles.tile([128, KVC, NTB, D], BF16)
            nc.sync.dma_start(
                out=v_sb, in_=v_d.ap().rearrange("h (sc p) d -> p h sc d", p=128)
            )
            mask_sb = singles.tile([128, SUMW], BF16)
            nc.sync.dma_start(out=mask_sb, in_=mask_d.ap())

            copy_ctr = 0
            for pair in range(PAIRS):
                kv = pair // G
                ptb = ptbuf_pool.tile([128, SUMW], BF16, tag="ptb")
                stage_m = stage_pool.tile([128, NTB], F32, tag="stm")
                stage_s = stage_pool.tile([128, NTB], F32, tag="sts")

                # ---- phase 1: scores, stats, exp, transpose ----
                for tb in range(NTB):
                    lo, w = s_lo[tb], W[tb]
                    npieces = (w + PIECE - 1) // PIECE
                    pieces = []
                    for pi in range(npieces):
                        p0 = pi * PIECE
                        pw = min(PIECE, w - p0)
                        sps = sps_pool.tile([128, PIECE], F32, tag="sps")
                        pieces.append((sps, p0, pw))
                        for c0 in range(0, pw, 512):
                            cw = min(512, pw - c0)
                            nc.tensor.matmul(
                                sps[:, c0:c0 + cw],
                                lhsT=qt_sb[:, pair, tb * 128:(tb + 1) * 128],
                                rhs=kt_sb[:, kv, lo + p0 + c0: lo + p0 + c0 + cw],
                                start=True, stop=False, skip_group_check=True,
                            )
                        for c0 in range(0, pw, 512):
                            cw = min(512, pw - c0)
                            mo = mask_off[tb] + p0 + c0
                            nc.tensor.matmul(
                                sps[:, c0:c0 + cw],
                                lhsT=ident,
                                rhs=mask_sb[:, mo:mo + cw],
                                start=False, stop=True, skip_group_check=True,
                            )
                        m_out = (
                            stage_m[:, tb:tb + 1] if npieces == 1
                            else stats_pool.tile([128, 1], F32, tag="mtmp")
                        )
                        nc.vector.tensor_reduce(
                            out=m_out, in_=sps[:, :pw],
                            op=mybir.AluOpType.max, axis=mybir.AxisListType.X,
                        )
                        pieces[-1] = (sps, p0, pw, m_out)
                    if npieces > 1:
                        nc.vector.tensor_tensor(
                            out=stage_m[:, tb:tb + 1],
                            in0=pieces[0][3], in1=pieces[1][3],
                            op=mybir.AluOpType.max,
                        )
                    bias_t = stats_pool.tile([128, 1], F32, tag="bias")
                    nc.vector.tensor_scalar_mul(bias_t, stage_m[:, tb:tb + 1], -LN)
                    pstrip = p_pool.tile([128, 2048], BF16, tag="p")
                    sum_parts = []
                    for sps, p0, pw, *_ in pieces:
                        s_out = (
                            stage_s[:, tb:tb + 1] if npieces == 1
                            else stats_pool.tile([128, 1], F32, tag="stmp")
                        )
                        nc.scalar.activation(
                            out=pstrip[:, p0:p0 + pw],
                            in_=sps[:, :pw],
                            func=mybir.ActivationFunctionType.Exp,
                            bias=bias_t,
                            scale=LN,
                            accum_out=s_out,
                        )
                        sum_parts.append(s_out)
                    if npieces > 1:
                        nc.vector.tensor_tensor(
                            out=stage_s[:, tb:tb + 1],
                            in0=sum_parts[0], in1=sum_parts[1],
                            op=mybir.AluOpType.add,
                        )
                    for i in range(w // 128):
                        sc = lo // 128 + i
                        ptp = ptps_pool.tile([128, 128], BF16, tag="ptps")
                        nc.tensor.transpose(ptp, pstrip[:, i * 128:(i + 1) * 128], ident)
                        dst = ptb[:, pt_off[(tb, sc)]:pt_off[(tb, sc)] + 128]
                        if copy_ctr % 3 == 2:
                            nc.scalar.copy(dst, ptp)
                        else:
                            nc.vector.tensor_copy(dst, ptp)
                        copy_ctr += 1

                # ---- phase 2: PV ----
                for tg in range(4):
                    otp = otps_pool.tile([128, 512], F32, tag="otps")
                    for j in range(4):
                        tb = tg * 4 + j
                        scs = list(range(s_lo[tb] // 128, tb + 1))
                        for si, sc in enumerate(scs):
                            nc.tensor.matmul(
                                otp[:, j * 128:(j + 1) * 128],
                                lhsT=v_sb[:, kv, sc, :],
                                rhs=ptb[:, pt_off[(tb, sc)]:pt_off[(tb, sc)] + 128],
                                start=(si == 0), stop=(si == len(scs) - 1),
                            )
                    ot_sb = otsb_pool.tile([128, 512], F32, tag="otsb")
                    if tg % 2 == 0:
                        nc.vector.tensor_copy(ot_sb, otp)
                    else:
                        nc.scalar.copy(ot_sb, otp)
                    nc.sync.dma_start(
                        out=o_d.ap()[pair, :, tg * 512:(tg + 1) * 512], in_=ot_sb
                    )

                nc.sync.dma_start(
                    out=om_d.ap().rearrange("h tb p -> p h tb")[:, pair, :], in_=stage_m
                )
                nc.sync.dma_start(
                    out=os_d.ap().rearrange("h tb p -> p h tb")[:, pair, :], in_=stage_s
                )

    nc.compile()
    return nc


def _prep_inputs(query, key, value, decoder_segment_ids, plan):
    """Build the 8 per-core input maps (host-side shard + layout)."""
    query = np.asarray(query, dtype=np.float32)
    key = np.asarray(key, dtype=np.float32)
    value = np.asarray(value, dtype=np.float32)
    seg = np.asarray(decoder_segment_ids)
    in_maps = []
    masks = [_masks_for_batch(seg[b], plan) for b in range(B)]
    for c in range(NCORES):
        bi, kp = c // 4, c % 4
        heads = [4 * kp + p for p in range(PAIRS)]
        kvs = [2 * kp + j for j in range(KVC)]
        qt = np.ascontiguousarray(
            query[bi][:, heads, :].transpose(1, 2, 0))          # [PAIRS, D, T]
        kt = np.ascontiguousarray(
            key[bi][:, kvs, :].transpose(1, 2, 0))              # [KVC, D, T]
        v = np.ascontiguousarray(
            value[bi][:, kvs, :].transpose(1, 0, 2)).astype(BF_NP)  # [KVC, T, D]
        in_maps.append({"qt": qt, "kt": kt, "v": v, "mask": masks[bi]})
    return in_maps


def _assemble(results):
    out = np.empty((B, T, NH, D), dtype=np.float32)
    mx = np.empty((B, T, NH, 1), dtype=np.float32)
    sm = np.empty((B, T, NH, 1), dtype=np.float32)
    for c in range(NCORES):
        bi, kp = c // 4, c % 4
        r = results[c]
        for p in range(PAIRS):
            h = 4 * kp + p
            out[bi, :, h, :] = r["o"][p].T
            mx[bi, :, h, 0] = r["om"][p].reshape(T)
            sm[bi, :, h, 0] = r["os"][p].reshape(T)
    return out, mx, sm


def build_for_inputs(query, key, value, decoder_segment_ids, num_kv_heads=NKV):
    """Returns (nc, in_maps, assemble_fn) — used by kernel() and test drivers."""
    assert int(num_kv_heads) == NKV
    plan = _plan(decoder_segment_ids)
    nc = _build_graph(plan)
    in_maps = _prep_inputs(query, key, value, decoder_segment_ids, plan)
    return nc, in_maps, _assemble


def kernel(query, key, value, decoder_segment_ids, num_kv_heads=NKV):
    nc, in_maps, assemble = build_for_inputs(
        query, key, value, decoder_segment_ids, num_kv_heads
    )
    res = run_bass_kernel_spmd(nc, in_maps, core_ids=list(range(NCORES)))
    return assemble(res.results)


# revision 24
# speedup vs baseline: 377.8383x; 377.8383x over previous
"""Trainium2 Bass kernel for GQA flash-attention partials (sparse_attention).

Problem: query [2,2048,16,128] f32, key/value [2,2048,8,128] f32,
decoder_segment_ids [2,2048] int32 (sorted), num_kv_heads=8.
Returns (local_out [b,t,n,d], local_max [b,t,n,1], local_sum [b,t,n,1]):
  S = einsum('btkgd,bskd->bkgts', q, k) + additive mask (segment & causal)
  local_max = rowmax(S); local_exps = 2.71828**(S - max); local_sum = rowsum
  local_out = einsum('bkgts,bskd->btkgd', local_exps, v)

Sharding: 8 cores; core c -> batch c//4, kv heads {2*(c%4), 2*(c%4)+1}
(4 query heads per core). No collectives needed; gather on host.

Per-core device pipeline (orientation: S[t_partitions, s_free]):
  - host pre-transposes Q,K to [d, t] layout; V stays [s, d] (cast to bf16)
  - per (pair, t-block): QK matmuls (float32r) -> PSUM strip [128, W]
    where the strip [s_lo, t_end) skips earlier segments (block-aligned)
  - DVE tensor_tensor_reduce: masked = S + mask(bf16), accum rowmax
  - ScalarE activation(Exp, bias=-ln(2.71828)*max, scale=ln(2.71828)),
    accum_out = rowsum; P written bf16
  - PE transpose of P chunks + DVE/ACT copies -> PT buffer (bf16)
  - PV: out^T[d, t] accumulated in PSUM over s-chunks (bf16 matmuls)
  - host transposes out^T back
"""

import numpy as np
import ml_dtypes

import concourse.bass as bass
import concourse.tile as tile
import concourse.mybir as mybir
from concourse import bacc
from concourse.bass_utils import run_bass_kernel_spmd
from concourse.masks import make_identity

B, T, NH, NKV, D = 2, 2048, 16, 8, 128
G = NH // NKV            # 2 query heads per kv head
NCORES = 8
KVC = NKV // 4           # 2 kv heads per core
PAIRS = KVC * G          # 4 query heads per core
NTB = T // 128           # 16 t-blocks
PIECE = 512             # max PSUM strip piece width (2 banks f32)

LN = float(np.log(np.float64(np.float32(2.71828))))   # exp base scale
MASK_VAL = float(-0.7 * np.finfo(np.float32).max)
NEG_BIG = -3.0e38

F32 = mybir.dt.float32
F32R = mybir.dt.float32r
BF16 = mybir.dt.bfloat16
BF_NP = ml_dtypes.bfloat16


def _plan(seg_ids):
    """Strip extents per t-block, unioned over batches (graph is SPMD-shared).

    Returns dict with per-tb s_lo (128-aligned), W, PT-buffer column offsets
    per (tb, sc) chunk, total column count SUMW, and which QK chunks need a
    mask-add matmul (union over batches).
    """
    seg_ids = np.asarray(seg_ids)
    # first index of the segment containing row r, per batch
    s_lo = []
    for tb in range(NTB):
        r0 = tb * 128
        lo = r0
        for b in range(seg_ids.shape[0]):
            seg = seg_ids[b]
            first = int(np.searchsorted(seg, seg[r0], side="left"))
            lo = min(lo, first)
        s_lo.append((lo // 128) * 128)
    W = [(tb + 1) * 128 - s_lo[tb] for tb in range(NTB)]
    pt_off = {}
    off = 0
    for tb in range(NTB):
        for sc in range(s_lo[tb] // 128, tb + 1):
            pt_off[(tb, sc)] = off
            off += 128
    mask_off = {}
    moff = 0
    for tb in range(NTB):
        mask_off[tb] = moff
        moff += W[tb]
    assert moff == off
    plan = dict(s_lo=s_lo, W=W, pt_off=pt_off, mask_off=mask_off, SUMW=moff)
    # which (tb, chunk-start) QK chunks contain any masked cell in any batch
    need = {}
    for b in range(seg_ids.shape[0]):
        m = _masks_for_batch(seg_ids[b], plan).astype(np.float32)
        for tb in range(NTB):
            w = W[tb]
            for pi in range((w + PIECE - 1) // PIECE):
                p0 = pi * PIECE
                pw = min(PIECE, w - p0)
                for c0 in range(0, pw, 512):
                    cw = min(512, pw - c0)
                    col = mask_off[tb] + p0 + c0
                    if np.any(m[:, col:col + cw] != 0.0):
                        need[(tb, p0 + c0)] = True
    plan["mask_needed"] = need
    return plan


def _masks_for_batch(seg, plan):
    """[128, SUMW] additive mask (0 / MASK_VAL) in bf16 for one batch."""
    seg = np.asarray(seg)
    cols = np.empty((128, plan["SUMW"]), dtype=np.float32)
    for tb in range(NTB):
        lo, w = plan["s_lo"][tb], plan["W"][tb]
        rows = np.arange(tb * 128, (tb + 1) * 128)
        cc = np.arange(lo, lo + w)
        ok = (seg[cc][None, :] == seg[rows][:, None]) & (cc[None, :] <= rows[:, None])
        cols[:, plan["mask_off"][tb]:plan["mask_off"][tb] + w] = np.where(ok, 0.0, MASK_VAL)
    return cols.astype(BF_NP)


def _build_graph(plan, loop_n=1, mode="full"):
    nc = bacc.Bacc("TRN2", target_bir_lowering=False, debug=False, num_devices=NCORES)

    qt_d = nc.declare_dram_parameter("qt", [PAIRS, D, T], F32R, isOutput=False)
    kt_d = nc.declare_dram_parameter("kt", [KVC, D, T], F32R, isOutput=False)
    v_d = nc.declare_dram_parameter("v", [KVC, T, D], BF16, isOutput=False)
    mask_d = nc.declare_dram_parameter("mask", [128, plan["SUMW"]], BF16, isOutput=False)
    o_d = nc.declare_dram_parameter("o", [PAIRS, D, T], F32, isOutput=True)
    om_d = nc.declare_dram_parameter("om", [PAIRS, NTB, 128], F32, isOutput=True)
    os_d = nc.declare_dram_parameter("os", [PAIRS, NTB, 128], F32, isOutput=True)

    s_lo, W, pt_off, mask_off = plan["s_lo"], plan["W"], plan["pt_off"], plan["mask_off"]
    SUMW = plan["SUMW"]
    pt_bufs = 2 if SUMW <= 12 * 1024 else 1

    with tile.TileContext(nc) as tc:
        with (
            tc.tile_pool(name="singles", bufs=1) as singles,
            tc.tile_pool(name="ptbuf", bufs=pt_bufs) as ptbuf_pool,
            tc.tile_pool(name="p", bufs=3) as p_pool,
            tc.tile_pool(name="stats", bufs=8) as stats_pool,
            tc.tile_pool(name="stage", bufs=4) as stage_pool,
            tc.tile_pool(name="otsb", bufs=2) as otsb_pool,
            tc.tile_pool(name="sps", bufs=4, space="PSUM") as sps_pool,
            tc.tile_pool(name="ptps", bufs=2, space="PSUM") as ptps_pool,
            tc.tile_pool(name="otps", bufs=2, space="PSUM") as otps_pool,
        ):
            ident = singles.tile([128, 128], BF16)
            make_identity(nc, ident)

            qt_sb = [singles.tile([128, T], F32R, name=f"qt{h}") for h in range(PAIRS)]
            kt_sb = [singles.tile([128, T], F32R, name=f"kt{h}") for h in range(KVC)]
            v_sb = [singles.tile([128, NTB, D], BF16, name=f"v{h}") for h in range(KVC)]
            mask_sb = [singles.tile([128, W[tb]], BF16, name=f"msk{tb}")
                       for tb in range(NTB)]

            def load(_iv=None):
                _emit_loads(nc, qt_sb, kt_sb, v_sb, mask_sb,
                            qt_d, kt_d, v_d, mask_d)

            stages = mode.split("_")[1] if "_" in mode else "all"

            def compute(_iv=None):
                _emit_body(nc, tc, plan, pools, qt_sb, kt_sb, v_sb, mask_sb, ident,
                           o_d, om_d, os_d, stages=stages)

            def body(_iv=None):
                load(_iv)
                compute(_iv)

            pools = dict(
                ptbuf_pool=ptbuf_pool, p_pool=p_pool, stats_pool=stats_pool,
                stage_pool=stage_pool, otsb_pool=otsb_pool, sps_pool=sps_pool,
                ptps_pool=ptps_pool, otps_pool=otps_pool,
            )
            base = mode.split("_")[0]
            fn = {"full": body, "dma": load, "compute": compute}[base]
            if base == "compute":
                load()
            if loop_n > 1:
                with tc.For_i(0, loop_n, 1) as _i:
                    fn(_i)
            else:
                fn()

    nc.compile()
    return nc


def _emit_loads(nc, qt_sb, kt_sb, v_sb, mask_sb, qt_d, kt_d, v_d, mask_d):
    """Per-slice destination tiles so consumers wait only on their own loads;
    each tensor still split into multiple DMAs for SDMA parallelism."""
    qt_ap = qt_d.ap().rearrange("h d t -> d h t")
    for h in range(PAIRS):
        for half in range(2):
            sl = slice(half * (T // 2), (half + 1) * (T // 2))
            nc.sync.dma_start(out=qt_sb[h][:, sl], in_=qt_ap[:, h, sl])
    kt_ap = kt_d.ap().rearrange("h d t -> d h t")
    for h in range(KVC):
        for half in range(2):
            sl = slice(half * (T // 2), (half + 1) * (T // 2))
            nc.sync.dma_start(out=kt_sb[h][:, sl], in_=kt_ap[:, h, sl])
    v_ap = v_d.ap().rearrange("h (sc p) d -> p h sc d", p=128)
    for h in range(KVC):
        nc.sync.dma_start(out=v_sb[h], in_=v_ap[:, h])
    moffs = mask_sb[0].nc_plan_moffs if hasattr(mask_sb[0], "nc_plan_moffs") else None
    off = 0
    for tb in range(NTB):
        w = mask_sb[tb].shape[-1]
        nc.sync.dma_start(out=mask_sb[tb], in_=mask_d.ap()[:, off:off + w])
        off += w


def _emit_body(nc, tc, plan, pools, qt_sb, kt_sb, v_sb, mask_sb, ident,
               o_d, om_d, os_d, stages="all"):
    """Software-pipelined emission: per (pair) phase-1 items are strip-pieces;
    QK runs LAG items ahead of reduce/exp/transpose so no engine's static
    instruction order blocks on a later-stage semaphore. Phase-2 (PV) for
    pair p is emitted during phase-1 of pair p+1."""
    do_exp = stages in ("qke", "qket", "all")
    do_pt = stages in ("qket", "all")
    do_pv = stages == "all"
    s_lo, W, pt_off, mask_off = plan["s_lo"], plan["W"], plan["pt_off"], plan["mask_off"]
    SUMW = plan["SUMW"]
    ptbuf_pool = pools["ptbuf_pool"]
    p_pool = pools["p_pool"]
    stats_pool = pools["stats_pool"]
    stage_pool = pools["stage_pool"]
    otsb_pool = pools["otsb_pool"]
    sps_pool = pools["sps_pool"]
    ptps_pool = pools["ptps_pool"]
    otps_pool = pools["otps_pool"]

    # per-pair mutable state
    state = {}

    def items_of(pair):
        out = []
        for tb in range(NTB):
            w = W[tb]
            npieces = (w + PIECE - 1) // PIECE
            for pi in range(npieces):
                out.append((tb, pi, npieces))
        return out

    def emit_qk(pair, it):
        kv = pair // G
        tb, pi, npieces = it
        lo, w = s_lo[tb], W[tb]
        p0 = pi * PIECE
        pw = min(PIECE, w - p0)
        sps = sps_pool.tile([128, PIECE], F32, tag="sps", name="sps")
        state[(pair, tb, pi)] = sps
        
        mask_chunks = []
        for c0 in range(0, pw, 512):
            cw = min(512, pw - c0)
            needs_mask = plan["mask_needed"].get((tb, p0 + c0), False)
            nc.tensor.matmul(
                sps[:, c0:c0 + cw],
                lhsT=qt_sb[pair][:, tb * 128:(tb + 1) * 128],
                rhs=kt_sb[kv][:, lo + p0 + c0: lo + p0 + c0 + cw],
                start=True, stop=not needs_mask, skip_group_check=True,
            )
            if needs_mask:
                mask_chunks.append((c0, cw))
        for c0, cw in mask_chunks:
            mo = p0 + c0
            nc.tensor.matmul(
                sps[:, c0:c0 + cw], lhsT=ident, rhs=mask_sb[tb][:, mo:mo + cw],
                start=False, stop=True, skip_group_check=True,
            )

    def emit_stats(pair, it):
        tb, pi, npieces = it
        stage_m = state[(pair, "stm")]
        sps = state[(pair, tb, pi)]
        w = W[tb]
        pw = min(PIECE, w - pi * PIECE)
        last = pi == npieces - 1
        if npieces == 1:
            m_out = stage_m[:, tb:tb + 1]
        else:
            m_out = stats_pool.tile([128, 1], F32, tag="mtmp")
            state[(pair, tb, pi, "m")] = m_out
        nc.vector.tensor_reduce(
            out=m_out, in_=sps[:, :pw],
            op=mybir.AluOpType.max, axis=mybir.AxisListType.X,
        )
        if last and npieces > 1:
            nc.vector.tensor_tensor(
                out=stage_m[:, tb:tb + 1],
                in0=state[(pair, tb, 0, "m")], in1=m_out,
                op=mybir.AluOpType.max,
            )
        if last and do_exp:
            bias_t = stats_pool.tile([128, 1], F32, tag="bias")
            nc.vector.tensor_scalar_mul(bias_t, stage_m[:, tb:tb + 1], -LN)
            state[(pair, tb, "bias")] = bias_t

    def emit_exp(pair, it):
        if not do_exp:
            return
        tb, pi, npieces = it
        stage_s = state[(pair, "sts")]
        sps = state[(pair, tb, pi)]
        w = W[tb]
        p0 = pi * PIECE
        pw = min(PIECE, w - p0)
        if pi == 0:
            state[(pair, tb, "p")] = p_pool.tile([128, 2048], BF16, tag="p", name="pstrip")
        pstrip = state[(pair, tb, "p")]
        if npieces == 1:
            s_out = stage_s[:, tb:tb + 1]
        else:
            s_out = stats_pool.tile([128, 1], F32, tag="stmp")
            state[(pair, tb, pi, "s")] = s_out
        nc.scalar.activation(
            out=pstrip[:, p0:p0 + pw], in_=sps[:, :pw],
            func=mybir.ActivationFunctionType.Exp,
            bias=state[(pair, tb, "bias")], scale=LN, accum_out=s_out,
        )
        if npieces > 1 and pi == npieces - 1:
            nc.vector.tensor_tensor(
                out=stage_s[:, tb:tb + 1],
                in0=state[(pair, tb, 0, "s")], in1=s_out,
                op=mybir.AluOpType.add,
            )

    def emit_pt(pair, it):
        if not do_pt:
            return
        tb, pi, npieces = it
        lo, w = s_lo[tb], W[tb]
        p0 = pi * PIECE
        pw = min(PIECE, w - p0)
        pstrip = state[(pair, tb, "p")]
        ptb = state[(pair, "ptb")]
        nchunks = pw // 128
        for g0 in range(0, nchunks, 4):
            gn = min(4, nchunks - g0)
            ptp = ptps_pool.tile([128, 512], BF16, tag="ptps")
            for q in range(gn):
                i = p0 // 128 + g0 + q
                nc.tensor.transpose(
                    ptp[:, q * 128:(q + 1) * 128],
                    pstrip[:, i * 128:(i + 1) * 128], ident,
                )
            sc0 = lo // 128 + p0 // 128 + g0
            dst = ptb[:, pt_off[(tb, sc0)]:pt_off[(tb, sc0)] + gn * 128]
            nc.vector.tensor_copy(dst, ptp[:, :gn * 128])

    def phase1(pair):
        state[(pair, "ptb")] = ptbuf_pool.tile([128, SUMW], BF16, tag="ptb", name="ptb")
        state[(pair, "stm")] = stage_pool.tile([128, NTB], F32, tag="stm", name="stm")
        state[(pair, "sts")] = stage_pool.tile([128, NTB], F32, tag="sts", name="sts")
        items = items_of(pair)
        n = len(items)
        for i in range(n + 3):
            if i < n:
                emit_qk(pair, items[i])
            if 1 <= i + 0 and i - 1 < n and i >= 1:
                emit_stats(pair, items[i - 1])
            if i >= 2 and i - 2 < n:
                emit_exp(pair, items[i - 2])
            if i >= 3 and i - 3 < n:
                emit_pt(pair, items[i - 3])

    def phase2(pair):
        if not do_pv:
            return
        kv = pair // G
        ptb = state[(pair, "ptb")]
        for tg in range(4):
            otp = otps_pool.tile([128, 512], F32, tag="otps")
            for j in range(4):
                tb = tg * 4 + j
                scs = list(range(s_lo[tb] // 128, tb + 1))
                for si, sc in enumerate(scs):
                    nc.tensor.matmul(
                        otp[:, j * 128:(j + 1) * 128],
                        lhsT=v_sb[kv][:, sc, :],
                        rhs=ptb[:, pt_off[(tb, sc)]:pt_off[(tb, sc)] + 128],
                        start=(si == 0), stop=(si == len(scs) - 1),
                    )
            ot_sb = otsb_pool.tile([128, 512], F32, tag="otsb")
            nc.scalar.copy(ot_sb, otp)
            nc.sync.dma_start(
                out=o_d.ap()[pair, :, tg * 512:(tg + 1) * 512], in_=ot_sb
            )
        nc.sync.dma_start(
            out=om_d.ap().rearrange("h tb p -> p h tb")[:, pair, :],
            in_=state[(pair, "stm")],
        )
        nc.sync.dma_start(
            out=os_d.ap().rearrange("h tb p -> p h tb")[:, pair, :],
            in_=state[(pair, "sts")],
        )

    for pair in range(PAIRS):
        phase1(pair)
        if pair >= 1:
            phase2(pair - 1)
    phase2(PAIRS - 1)


def _prep_inputs(query, key, value, decoder_segment_ids, plan):
    """Build the 8 per-core input maps (host-side shard + layout)."""
    query = np.asarray(query, dtype=np.float32)
    key = np.asarray(key, dtype=np.float32)
    value = np.asarray(value, dtype=np.float32)
    seg = np.asarray(decoder_segment_ids)
    in_maps = []
    masks = [_masks_for_batch(seg[b], plan) for b in range(B)]
    for c in range(NCORES):
        bi, kp = c // 4, c % 4
        heads = [4 * kp + p for p in range(PAIRS)]
        kvs = [2 * kp + j for j in range(KVC)]
        qt = np.ascontiguousarray(
            query[bi][:, heads, :].transpose(1, 2, 0))          # [PAIRS, D, T]
        kt = np.ascontiguousarray(
            key[bi][:, kvs, :].transpose(1, 2, 0))              # [KVC, D, T]
        v = np.ascontiguousarray(
            value[bi][:, kvs, :].transpose(1, 0, 2)).astype(BF_NP)  # [KVC, T, D]
        in_maps.append({"qt": qt, "kt": kt, "v": v, "mask": masks[bi]})
    return in_maps


def _assemble(results):
    out = np.empty((B, T, NH, D), dtype=np.float32)
    mx = np.empty((B, T, NH, 1), dtype=np.float32)
    sm = np.empty((B, T, NH, 1), dtype=np.float32)
    for c in range(NCORES):
        bi, kp = c // 4, c % 4
        r = results[c]
        for p in range(PAIRS):
            h = 4 * kp + p
            out[bi, :, h, :] = r["o"][p].T
            mx[bi, :, h, 0] = r["om"][p].reshape(T)
            sm[bi, :, h, 0] = r["os"][p].reshape(T)
    return out, mx, sm


def build_for_inputs(query, key, value, decoder_segment_ids, num_kv_heads=NKV):
    """Returns (nc, in_maps, assemble_fn) — used by kernel() and test drivers."""
    assert int(num_kv_heads) == NKV
    plan = _plan(decoder_segment_ids)
    nc = _build_graph(plan)
    in_maps = _prep_inputs(query, key, value, decoder_segment_ids, plan)
    return nc, in_maps, _assemble


def kernel(query, key, value, decoder_segment_ids, num_kv_heads=NKV):
    nc, in_maps, assemble = build_for_inputs(
        query, key, value, decoder_segment_ids, num_kv_heads
    )
    res = run_bass_kernel_spmd(nc, in_maps, core_ids=list(range(NCORES)))
    return assemble(res.results)
